# revision 7
# baseline (speedup 1.0000x reference)
"""HeadQK kernel for trn2: out = segsum_vocab(causal(q @ k.T / 256)) over 8 cores.

Strategy: cover the causally-active (j-tile, t-chunk) blocks of the T x T
attention matrix c with 8 uniform regions, one per core.  Each region is
NQ=3 t-chunks x NK=8 j-tiles (24 blocks); a block computes
c[j, t] = <k_j, q_t> with k = x @ Wk, q = x @ (Wq/256).  A core computes q
only for its region's 3 chunks and k only for its 8 j-tiles, so the big
projection work is split across cores instead of replicated.

This version is scheduled for PE occupancy:
 - inputs stream in need-ordered ~256KB pieces so the first matmul starts
   ~2us in instead of waiting for whole-tensor DMAs;
 - the k pass is c8-outer (d,h inner) so it consumes the x stream at half
   pace and never outruns the DMA;
 - stationary weights are reused across the h/i inner loops (48 LDWEIGHTS
   instead of 128);
 - PSUM->SBUF casts round-robin over vector/scalar/gpsimd and output DMAs
   alternate between the SP and Activation HWDGE queues so neither the
   casts nor the ~600ns DMA-issue cost ever gates the PE;
 - each c block is written as a contiguous [128,512] bf16 DRAM blob.
The host applies the causal tril mask and the vocab segment-sum in fp32.
The device program is identical on every core (SPMD); per-core work
differs only through input data.
"""

import sys

import numpy as np

if "/opt/trn_rl_repo" not in sys.path:
    sys.path.insert(0, "/opt/trn_rl_repo")

import ml_dtypes

import concourse.bacc as bacc
import concourse.mybir as mybir
import concourse.tile as tile
from concourse.bass_utils import run_bass_kernel_spmd

T, C, D, V = 4096, 1024, 256, 32000
NCORES = 8
NCH = 8            # t chunks in T
CW = T // NCH      # 512
NQ = 3             # t-chunks per region
NK = 8             # j-tiles per region
HK = NK // 2       # j-tiles per xkt half
CT = C // 128      # 8 contraction tiles
DT = D // 128      # 2 d tiles
F32 = mybir.dt.float32
BF16 = mybir.dt.bfloat16
BF = ml_dtypes.bfloat16

# core p computes blocks (g, ch) for ch in REGIONS[p][0], g in REGIONS[p][1];
# together the regions cover every causally-active block (ch >= g//4).
REGIONS = [
    ([7, 6, 5], [0, 1, 2, 3, 4, 5, 6, 7]),
    ([7, 6, 5], [8, 9, 10, 11, 12, 13, 14, 15]),
    ([7, 6, 5], [16, 17, 18, 19, 20, 21, 22, 23]),
    ([7, 6, 4], [24, 25, 26, 27, 28, 29, 30, 31]),
    ([4, 3, 2], [0, 1, 2, 3, 4, 5, 6, 7]),
    ([4, 3, 2], [8, 9, 10, 11, 12, 13, 14, 15]),
    ([4, 1, 0], [16, 17, 18, 19, 4, 5, 6, 7]),
    ([1, 0, 3], [0, 1, 2, 3, 4, 5, 6, 7]),
]


def _build():
    nc = bacc.Bacc("TRN2", target_bir_lowering=False, debug=False,
                   num_devices=NCORES)
    # wk pieces: [128, 4*256], c8-major: wk[cin, c8*256 + d*128 + col]
    wka = nc.dram_tensor("wka", [128, 4 * 256], BF16, kind="ExternalInput")
    wkb = nc.dram_tensor("wkb", [128, 4 * 256], BF16, kind="ExternalInput")
    # xk pieces: [128, 2*CW] (2 c8 groups each)
    xkp = [[nc.dram_tensor(f"xk{h}p{j}", [128, 2 * CW], BF16,
                           kind="ExternalInput") for j in range(4)]
           for h in range(2)]
    # wq d-major: wq[cin, d*1024 + c8*128 + col]
    wq = nc.dram_tensor("wq", [128, DT * 1024], BF16, kind="ExternalInput")
    # xh pieces: [128, 4*CW] (c8 0-3 / 4-7) per chunk
    xhp = [[nc.dram_tensor(f"xh{i}{s}", [128, 4 * CW], BF16,
                           kind="ExternalInput") for s in range(2)]
           for i in range(NQ)]
    out = nc.dram_tensor("out", [NK, 128, NQ * CW], BF16,
                         kind="ExternalOutput")

    with tile.TileContext(nc) as tc:
        with (
            tc.tile_pool(name="const", bufs=1) as cpool,
            tc.tile_pool(name="obuf", bufs=6) as opool,
            tc.tile_pool(name="psqk", bufs=4, space="PSUM") as psqk,
            tc.tile_pool(name="psc", bufs=4, space="PSUM") as psc,
        ):
            # --- input DMAs, need order (all on the SP HWDGE queue) ---
            wka_b = cpool.tile([128, 4 * 256], BF16, tag="wka")
            nc.sync.dma_start(out=wka_b[:], in_=wka[:])
            xk_b = [[cpool.tile([128, 2 * CW], BF16, tag=f"xk{h}p{j}",
                                name=f"xk{h}p{j}b")
                     for j in range(4)] for h in range(2)]
            for j in range(2):
                for h in range(2):
                    nc.sync.dma_start(out=xk_b[h][j][:], in_=xkp[h][j][:])
            wkb_b = cpool.tile([128, 4 * 256], BF16, tag="wkb")
            nc.sync.dma_start(out=wkb_b[:], in_=wkb[:])
            for j in range(2, 4):
                for h in range(2):
                    nc.sync.dma_start(out=xk_b[h][j][:], in_=xkp[h][j][:])
            wq_b = cpool.tile([128, DT * 1024], BF16, tag="wq")
            nc.sync.dma_start(out=wq_b[:], in_=wq[:])
            xh_b = [[cpool.tile([128, 4 * CW], BF16, tag=f"xh{i}{s}",
                                name=f"xh{i}{s}b")
                     for s in range(2)] for i in range(NQ)]
            for s in range(2):
                for i in range(NQ):
                    nc.sync.dma_start(out=xh_b[i][s][:], in_=xhp[i][s][:])

            # cast engine round-robin: vector / scalar (gpsimd can't see PSUM)
            ncast = [0]

            def psum_to_sbuf(dst, src):
                if ncast[0] % 2 == 0:
                    nc.vector.tensor_copy(out=dst, in_=src)
                else:
                    nc.scalar.copy(out=dst, in_=src)
                ncast[0] += 1

            # --- k pass: c8-outer so PE tracks the input stream; two
            # passes (d0 then d1) so only 2 PSUM banks stay open at a time
            # ktb[d][dp, tt*128 + jj] = k[j-tile tt, j=jj, d*128 + dp]
            ktb = [cpool.tile([128, NK * 128], BF16, tag=f"kt{d}", name=f"ktb{d}")
                   for d in range(DT)]
            for d in range(DT):
                kp = [psqk.tile([128, CW], F32, tag="qk", name=f"kp{_}")
                      for _ in range(2)]
                for c8 in range(CT):
                    wkt = wka_b if c8 < 4 else wkb_b
                    co = (c8 % 4) * 256
                    for h in range(2):
                        nc.tensor.matmul(
                            out=kp[h][:],
                            lhsT=wkt[:, co + d * 128:co + (d + 1) * 128],
                            rhs=xk_b[h][c8 // 2][:, (c8 % 2) * CW:
                                                 (c8 % 2 + 1) * CW],
                            start=(c8 == 0), stop=(c8 == CT - 1),
                        )
                for h in range(2):
                    psum_to_sbuf(ktb[d][:, h * CW:(h + 1) * CW], kp[h][:])

            # --- q pass: d-outer, i-inner (stationary reused 3x) ---
            qt = [cpool.tile([128, NQ * CW], BF16, tag=f"qt{d}", name=f"qtb{d}")
                  for d in range(DT)]
            for d in range(DT):
                qp = [psqk.tile([128, CW], F32, tag="qk", name=f"qp{_}") for _ in range(NQ)]
                for c8 in range(CT):
                    for i in range(NQ):
                        nc.tensor.matmul(
                            out=qp[i][:],
                            lhsT=wq_b[:, d * 1024 + c8 * 128:
                                      d * 1024 + (c8 + 1) * 128],
                            rhs=xh_b[i][c8 // 4][:, (c8 % 4) * CW:
                                                 (c8 % 4 + 1) * CW],
                            start=(c8 == 0), stop=(c8 == CT - 1),
                        )
                for i in range(NQ):
                    psum_to_sbuf(qt[d][:, i * CW:(i + 1) * CW], qp[i][:])

            # --- c blocks: per j-tile, d-outer / i-inner (ktb reused 3x);
            # casts alternate vector/scalar; per tile the i=0 block goes out
            # alone and i=1,2 go out as one contiguous 2KB-line DMA, so all
            # 16 output issues fit on the SP HWDGE with headroom
            for tt in range(NK):
                cp = [psc.tile([128, CW], F32, tag="cp", name=f"cp{_}") for _ in range(NQ)]
                for d in range(DT):
                    for i in range(NQ):
                        nc.tensor.matmul(
                            out=cp[i][:],
                            lhsT=ktb[d][:, tt * 128:(tt + 1) * 128],
                            rhs=qt[d][:, i * CW:(i + 1) * CW],
                            start=(d == 0), stop=(d == DT - 1),
                        )
                ob0 = opool.tile([128, CW], BF16, tag="ob0")
                psum_to_sbuf(ob0[:], cp[0][:])
                nc.sync.dma_start(out=out[tt][:, 0:CW], in_=ob0[:])
                ob12 = opool.tile([128, 2 * CW], BF16, tag="ob12")
                psum_to_sbuf(ob12[:, 0:CW], cp[1][:])
                psum_to_sbuf(ob12[:, CW:2 * CW], cp[2][:])
                nc.sync.dma_start(out=out[tt][:, CW:NQ * CW], in_=ob12[:])
    nc.compile()
    return nc


def kernel(x, idx, Wq, Wk):
    x = np.asarray(x, dtype=np.float32)
    idx = np.asarray(idx).astype(np.int64)
    Wq = np.asarray(Wq, dtype=np.float32)
    Wk = np.asarray(Wk, dtype=np.float32)

    xb = x.astype(BF)
    # xh_all[ch, cin, c8*CW + tin] = x[ch*CW + tin, c8*128 + cin]
    xh_all = np.ascontiguousarray(
        xb.reshape(NCH, CW, CT, 128).transpose(0, 3, 2, 1)
        .reshape(NCH, 128, CT * CW))
    # wq d-major: wqD[cin, d*1024 + c8*128 + col]
    wqD = np.ascontiguousarray(
        (Wq / 256.0).astype(BF).reshape(CT, 128, DT, 128)
        .transpose(1, 2, 0, 3).reshape(128, DT * 1024))
    # wk c8-major: wk2[cin, c8*256 + d*128 + col]
    wk2 = np.ascontiguousarray(
        Wk.astype(BF).reshape(CT, 128, D).transpose(1, 0, 2)
        .reshape(128, CT * D))

    in_maps = []
    for p in range(NCORES):
        chunks, tiles = REGIONS[p]
        m = {"wka": np.ascontiguousarray(wk2[:, :4 * 256]),
             "wkb": np.ascontiguousarray(wk2[:, 4 * 256:]),
             "wq": wqD}
        for h in range(2):
            rows = np.concatenate(
                [np.arange(g * 128, (g + 1) * 128)
                 for g in tiles[h * HK:(h + 1) * HK]])
            # xk[cin, c8*CW + tt*128 + jj] = x[rows[tt*128+jj], c8*128+cin]
            xs = xb[rows]                              # [HK*128, C]
            xk = np.ascontiguousarray(
                xs.reshape(HK * 128, CT, 128).transpose(2, 1, 0)
                .reshape(128, CT * HK * 128))
            for j in range(4):
                m[f"xk{h}p{j}"] = np.ascontiguousarray(
                    xk[:, j * 2 * CW:(j + 1) * 2 * CW])
        for i, ch in enumerate(chunks):
            m[f"xh{i}0"] = np.ascontiguousarray(xh_all[ch][:, :4 * CW])
            m[f"xh{i}1"] = np.ascontiguousarray(xh_all[ch][:, 4 * CW:])
        in_maps.append(m)

    nc = _build()
    res = run_bass_kernel_spmd(nc, in_maps, core_ids=list(range(NCORES)))

    # assemble c [T(j), T(t)] in fp32 from the active blocks of each region,
    # apply the causal mask, segment-sum over j -> vocab on the host
    cmat = np.zeros((T, T), np.float32)
    for p in range(NCORES):
        chunks, tiles = REGIONS[p]
        blk = np.asarray(res.results[p]["out"]).astype(np.float32)
        for tt, g in enumerate(tiles):
            for qq, ch in enumerate(chunks):
                if ch >= g // 4:     # causally active block
                    cmat[g * 128:(g + 1) * 128, ch * CW:(ch + 1) * CW] = \
                        blk[tt, :, qq * CW:(qq + 1) * CW]
    jj = np.arange(T)
    cmat *= jj[None, :] >= jj[:, None]      # keep t >= j
    order = np.argsort(idx, kind="stable")
    sidx = idx[order]
    starts = np.flatnonzero(np.r_[True, sidx[1:] != sidx[:-1]])
    red = np.add.reduceat(cmat[order], starts, axis=0)  # [nu, T]
    outf = np.zeros((T, V), np.float32)
    outf[:, sidx[starts]] = red.T
    return outf


# revision 9
# speedup vs baseline: 1.1276x; 1.1276x over previous
"""HeadQK kernel for trn2: out = segsum_vocab(causal(q @ k.T / 256)) over 8 cores.

Strategy: cover the causally-active (j-tile, t-chunk) blocks of the T x T
attention matrix c with 8 uniform regions, one per core.  Each region is
NQ=3 t-chunks x NK=8 j-tiles (24 blocks); a block computes
c[j, t] = <k_j, q_t> with k = x @ Wk, q = x @ (Wq/256).  A core computes q
only for its region's 3 chunks and k only for its 8 j-tiles, so the big
projection work is split across cores instead of replicated.

Scheduling notes (all matmuls are [128,128]x[128,512] bf16 chains at the
~213ns back-to-back PE issue rate; switching PSUM banks mid-chain costs
~46ns, so every accumulation runs as one contiguous chain):
 - inputs stream need-ordered over BOTH HWDGE groups (SP carries the
   k-gate wk/xk0 path, Activation carries xk1/wq/xh1 in parallel) so the
   ~0.4MB/us per-group DMA rate never starves the PE;
 - first pieces are small so the first chain starts ~3us in
   (DMA doorbell->data latency bound);
 - PSUM->SBUF casts alternate vector/scalar;
 - c blocks for one j-tile go out as a single contiguous-line [128,1536]
   DMA; the last tile is split per-block so the final issue trails the
   final matmul by only ~1.4us.
The host applies the causal tril mask and the vocab segment-sum in fp32.
The device program is identical on every core (SPMD); per-core work
differs only through input data.
"""

import sys

import numpy as np

if "/opt/trn_rl_repo" not in sys.path:
    sys.path.insert(0, "/opt/trn_rl_repo")

import ml_dtypes

import concourse.bacc as bacc
import concourse.mybir as mybir
import concourse.tile as tile
from concourse.bass_utils import run_bass_kernel_spmd

T, C, D, V = 4096, 1024, 256, 32000
NCORES = 8
NCH = 8            # t chunks in T
CW = T // NCH      # 512
NQ = 3             # t-chunks per region
NK = 8             # j-tiles per region
HK = NK // 2       # j-tiles per xkt half
CT = C // 128      # 8 contraction tiles
DT = D // 128      # 2 d tiles
F32 = mybir.dt.float32
BF16 = mybir.dt.bfloat16
BF = ml_dtypes.bfloat16

# core p computes blocks (g, ch) for ch in REGIONS[p][0], g in REGIONS[p][1];
# together the regions cover every causally-active block (ch >= g//4).
REGIONS = [
    ([7, 6, 5], [0, 1, 2, 3, 4, 5, 6, 7]),
    ([7, 6, 5], [8, 9, 10, 11, 12, 13, 14, 15]),
    ([7, 6, 5], [16, 17, 18, 19, 20, 21, 22, 23]),
    ([7, 6, 4], [24, 25, 26, 27, 28, 29, 30, 31]),
    ([4, 3, 2], [0, 1, 2, 3, 4, 5, 6, 7]),
    ([4, 3, 2], [8, 9, 10, 11, 12, 13, 14, 15]),
    ([4, 1, 0], [16, 17, 18, 19, 4, 5, 6, 7]),
    ([1, 0, 3], [0, 1, 2, 3, 4, 5, 6, 7]),
]

# input piece tables: (name, c8_lo, c8_hi)
WK_PIECES = (("wk0", 0, 2), ("wk1", 2, 8))
XK0_PIECES = (("xk0p0", 0, 2), ("xk0p1", 2, 4), ("xk0p2", 4, 8))
XK1_PIECES = (("xk1p0", 0, 4), ("xk1p1", 4, 8))


def _build():
    nc = bacc.Bacc("TRN2", target_bir_lowering=False, debug=False,
                   num_devices=NCORES)
    dram = {}
    for nm, lo, hi in WK_PIECES:
        dram[nm] = nc.dram_tensor(nm, [128, (hi - lo) * 256], BF16,
                                  kind="ExternalInput")
    for nm, lo, hi in XK0_PIECES + XK1_PIECES:
        dram[nm] = nc.dram_tensor(nm, [128, (hi - lo) * CW], BF16,
                                  kind="ExternalInput")
    dram["wq"] = nc.dram_tensor("wq", [128, CT * 256], BF16,
                                kind="ExternalInput")
    for i in range(NQ):
        for s in range(2):
            nm = f"xh{i}{s}"
            dram[nm] = nc.dram_tensor(nm, [128, 4 * CW], BF16,
                                      kind="ExternalInput")
    out = nc.dram_tensor("out", [NK, 128, NQ * CW], BF16,
                         kind="ExternalOutput")

    with tile.TileContext(nc) as tc:
        with (
            tc.tile_pool(name="const", bufs=1) as cpool,
            tc.tile_pool(name="obuf", bufs=3) as opool,
            tc.tile_pool(name="psqk", bufs=4, space="PSUM") as psqk,
            tc.tile_pool(name="psc", bufs=4, space="PSUM") as psc,
        ):
            sb = {}
            for nm in dram:
                if nm == "out":
                    continue
                sb[nm] = cpool.tile(list(dram[nm].shape), BF16, tag=nm,
                                    name=f"{nm}b")
            # SP group: the k gate (wk + xk0), then xh0, then xh2.
            # Activation group in parallel: xk1, wq, xh1.
            for nm in ("wk0", "xk0p0", "wk1", "xk0p1", "xk0p2",
                       "xh00", "xh01", "xh20", "xh21"):
                nc.sync.dma_start(out=sb[nm][:], in_=dram[nm][:])
            for nm in ("xk1p0", "xk1p1", "wq", "xh10", "xh11"):
                nc.scalar.dma_start(out=sb[nm][:], in_=dram[nm][:])

            def wk_slice(c8, d):
                nm, lo, _ = next(p for p in WK_PIECES if p[1] <= c8 < p[2])
                o = (c8 - lo) * 256 + d * 128
                return sb[nm][:, o:o + 128]

            def xk_slice(h, c8):
                pieces = XK0_PIECES if h == 0 else XK1_PIECES
                nm, lo, _ = next(p for p in pieces if p[1] <= c8 < p[2])
                o = (c8 - lo) * CW
                return sb[nm][:, o:o + CW]

            def xh_slice(i, c8):
                o = (c8 % 4) * CW
                return sb[f"xh{i}{c8 // 4}"][:, o:o + CW]

            # cast engine round-robin: vector / scalar
            ncast = [0]

            def psum_to_sbuf(dst, src):
                if ncast[0] % 2 == 0:
                    nc.vector.tensor_copy(out=dst, in_=src)
                else:
                    nc.scalar.copy(out=dst, in_=src)
                ncast[0] += 1

            # --- k pass: per-(h,d) chains; h0 (xk0) first
            # ktb[d][dp, tt*128 + jj] = k[j-tile tt, j=jj, d*128 + dp]
            ktb = [cpool.tile([128, NK * 128], BF16, tag=f"kt{d}",
                              name=f"ktb{d}") for d in range(DT)]
            for h in range(2):
                for d in range(DT):
                    kp = psqk.tile([128, CW], F32, tag="qk", name="kp")
                    for c8 in range(CT):
                        nc.tensor.matmul(
                            out=kp[:], lhsT=wk_slice(c8, d),
                            rhs=xk_slice(h, c8),
                            start=(c8 == 0), stop=(c8 == CT - 1),
                        )
                    psum_to_sbuf(ktb[d][:, h * CW:(h + 1) * CW], kp[:])

            # --- q pass: per-(i,d) chains, i outer so each xh chunk feeds
            # ~3.4us of PE work as it lands
            qt = [cpool.tile([128, NQ * CW], BF16, tag=f"qt{d}",
                             name=f"qtb{d}") for d in range(DT)]
            for i in range(NQ):
                for d in range(DT):
                    qp = psqk.tile([128, CW], F32, tag="qk", name="qp")
                    for c8 in range(CT):
                        nc.tensor.matmul(
                            out=qp[:],
                            lhsT=sb["wq"][:, c8 * 256 + d * 128:
                                          c8 * 256 + (d + 1) * 128],
                            rhs=xh_slice(i, c8),
                            start=(c8 == 0), stop=(c8 == CT - 1),
                        )
                    psum_to_sbuf(qt[d][:, i * CW:(i + 1) * CW], qp[:])

            # --- c blocks: per j-tile, per-block 2-matmul chains; one
            # [128, NQ*CW] output DMA per tile (3KB lines), last tile split
            # per block so the final issue trails the final matmul closely
            for tt in range(NK):
                cp = [psc.tile([128, CW], F32, tag="cp", name=f"cp{_}")
                      for _ in range(NQ)]
                for i in range(NQ):
                    for d in range(DT):
                        nc.tensor.matmul(
                            out=cp[i][:],
                            lhsT=ktb[d][:, tt * 128:(tt + 1) * 128],
                            rhs=qt[d][:, i * CW:(i + 1) * CW],
                            start=(d == 0), stop=(d == DT - 1),
                        )
                if tt < NK - 1:
                    ob = opool.tile([128, NQ * CW], BF16, tag="obf")
                    for i in range(NQ):
                        psum_to_sbuf(ob[:, i * CW:(i + 1) * CW], cp[i][:])
                    nc.sync.dma_start(out=out[tt][:], in_=ob[:])
                else:
                    for i in range(NQ):
                        obs = opool.tile([128, CW], BF16, tag=f"obs{i}",
                                         name=f"obs{i}")
                        psum_to_sbuf(obs[:], cp[i][:])
                        eng = nc.scalar if i == 1 else nc.sync
                        eng.dma_start(out=out[tt][:, i * CW:(i + 1) * CW],
                                      in_=obs[:])
    nc.compile()
    return nc


def kernel(x, idx, Wq, Wk):
    x = np.asarray(x, dtype=np.float32)
    idx = np.asarray(idx).astype(np.int64)
    Wq = np.asarray(Wq, dtype=np.float32)
    Wk = np.asarray(Wk, dtype=np.float32)

    xb = x.astype(BF)
    # xh_all[ch, cin, c8*CW + tin] = x[ch*CW + tin, c8*128 + cin]
    xh_all = np.ascontiguousarray(
        xb.reshape(NCH, CW, CT, 128).transpose(0, 3, 2, 1)
        .reshape(NCH, 128, CT * CW))
    # wq c8-major: wq2[cin, c8*256 + d*128 + col], scaled by 1/256
    wq2 = np.ascontiguousarray(
        (Wq / 256.0).astype(BF).reshape(CT, 128, D).transpose(1, 0, 2)
        .reshape(128, CT * D))
    # wk c8-major: wk2[cin, c8*256 + d*128 + col]
    wk2 = np.ascontiguousarray(
        Wk.astype(BF).reshape(CT, 128, D).transpose(1, 0, 2)
        .reshape(128, CT * D))

    in_maps = []
    for p in range(NCORES):
        chunks, tiles = REGIONS[p]
        m = {"wq": wq2}
        for nm, lo, hi in WK_PIECES:
            m[nm] = np.ascontiguousarray(wk2[:, lo * 256:hi * 256])
        for h in range(2):
            rows = np.concatenate(
                [np.arange(g * 128, (g + 1) * 128)
                 for g in tiles[h * HK:(h + 1) * HK]])
            # xk[cin, c8*CW + tt*128 + jj] = x[rows[tt*128+jj], c8*128+cin]
            xs = xb[rows]                              # [HK*128, C]
            xk = np.ascontiguousarray(
                xs.reshape(HK * 128, CT, 128).transpose(2, 1, 0)
                .reshape(128, CT * HK * 128))
            for nm, lo, hi in (XK0_PIECES if h == 0 else XK1_PIECES):
                m[nm] = np.ascontiguousarray(xk[:, lo * CW:hi * CW])
        for i, ch in enumerate(chunks):
            m[f"xh{i}0"] = np.ascontiguousarray(xh_all[ch][:, :4 * CW])
            m[f"xh{i}1"] = np.ascontiguousarray(xh_all[ch][:, 4 * CW:])
        in_maps.append(m)

    nc = _build()
    res = run_bass_kernel_spmd(nc, in_maps, core_ids=list(range(NCORES)))

    # assemble c [T(j), T(t)] in fp32 from the active blocks of each region,
    # apply the causal mask, segment-sum over j -> vocab on the host
    cmat = np.zeros((T, T), np.float32)
    for p in range(NCORES):
        chunks, tiles = REGIONS[p]
        blk = np.asarray(res.results[p]["out"]).astype(np.float32)
        for tt, g in enumerate(tiles):
            for qq, ch in enumerate(chunks):
                if ch >= g // 4:     # causally active block
                    cmat[g * 128:(g + 1) * 128, ch * CW:(ch + 1) * CW] = \
                        blk[tt, :, qq * CW:(qq + 1) * CW]
    jj = np.arange(T)
    cmat *= jj[None, :] >= jj[:, None]      # keep t >= j
    order = np.argsort(idx, kind="stable")
    sidx = idx[order]
    starts = np.flatnonzero(np.r_[True, sidx[1:] != sidx[:-1]])
    red = np.add.reduceat(cmat[order], starts, axis=0)  # [nu, T]
    outf = np.zeros((T, V), np.float32)
    outf[:, sidx[starts]] = red.T
    return outf
